# revision 1
# baseline (speedup 1.0000x reference)
"""Multi-head attention (B=4, S=2048, D=768, H=12, d=64) on 8 Trainium2 cores.

Sharding: core (b, g) = batch b in [0,4), head-group g in [0,2) — 6 heads each.
Each core computes the qkv projection for its heads, attention, and a partial
(transposed) output projection; the host sums the two head-group partials per
batch and adds b_proj.

Dataflow (per core, all matmuls in float32r = E8M11, full PE rate):
  - host passes x[b].T so contraction dims land on SBUF partitions
  - qkT[768,2048] = wqk.T @ xT   (q rows pre-scaled by 1/8 on host)
  - V[2048,768] = xT.T @ wv + bv, per-head layout [v_h | ones(64)]
  - per head pair, per 1024-col i-half, per 128-row j-tile:
      ST[j,i] = kT.T @ qT   (K=64 contraction; head parity alternates base
      partition 0/64 — alternating row-groups keeps the PE at full rate)
      pT = exp(ST)          (ACT, PSUM->SBUF, float32r out)
      av += [v_h | ones].T @ pT   (rows 0-63: out^T, rows 64-127: softmax
      denominator replicated — the ones-columns trick)
    aoT = av[0:64] * reciprocal(av[64:128])
  - outT[768,2048] = wp.T @ aoT  (partial; host: out[b] = outT_g0.T + outT_g1.T + b_proj)
"""
import numpy as np

B, S, D = 4, 2048, 768
H, DH = 12, 64
HPC = 6          # heads per core
NKT = D // 128   # 6 contraction tiles of 128
NSC = S // 512   # 4 column chunks of 512
NST = S // 128   # 16 row tiles of 128
NJT = 6          # qk projection output row tiles (768/128)
VW = HPC * 128  # v tile width: per head [v_h (64) | ones (64)]

_NC_CACHE = {}


def _round_fp32r(x):
    """Round fp32 to the fp32r grid (E8M11: low 12 mantissa bits zero, RNE)."""
    x = np.ascontiguousarray(x, dtype=np.float32)
    u = x.view(np.uint32).astype(np.uint64)
    u = (u + 0x7FF + ((u >> 12) & 1)) & 0xFFFFF000
    return u.astype(np.uint32).view(np.float32)


def _build_nc():
    import concourse.bass as bass
    import concourse.mybir as mybir
    import concourse.tile as tile
    from concourse import bacc

    f32r = mybir.dt.float32r
    f32 = mybir.dt.float32
    Exp = mybir.ActivationFunctionType.Exp

    nc = bacc.Bacc("TRN2", target_bir_lowering=False, debug=False)
    xT = nc.dram_tensor("xT", [D, S], f32r, kind="ExternalInput").ap()
    wqk = nc.dram_tensor("wqk", [D, 768], f32r, kind="ExternalInput").ap()
    bqk = nc.dram_tensor("bqk", [128, NJT], f32, kind="ExternalInput").ap()
    wv = nc.dram_tensor("wv", [D, 384], f32r, kind="ExternalInput").ap()
    bv = nc.dram_tensor("bv", [128, 384], f32, kind="ExternalInput").ap()
    wp = nc.dram_tensor("wp", [384, D], f32r, kind="ExternalInput").ap()
    outT = nc.dram_tensor("outT", [D, S], f32, kind="ExternalOutput").ap()

    with tile.TileContext(nc) as tc:
        with (
            tc.tile_pool(name="persist", bufs=1) as pp,
            tc.tile_pool(name="stage", bufs=6) as stg,
            tc.tile_pool(name="rec", bufs=4) as recp,
        ):
            # ---- persistent SBUF tensors ----
            qT_t = [pp.tile([128, S], f32r, name=f"qT{i}") for i in range(3)]
            kT_t = [pp.tile([128, S], f32r, name=f"kT{i}") for i in range(3)]
            v_t = [pp.tile([128, VW], f32r, name=f"v{i}") for i in range(NST)]
            bqk_t = pp.tile([128, NJT], f32, name="bqk")
            bv_t = pp.tile([128, 384], f32, name="bv")

            nc.sync.dma_start(bqk_t[:], bqk)
            nc.sync.dma_start(bv_t[:], bv)
            def v_strided(st, off):
                base = v_t[st][:]
                return bass.AP(
                    base.tensor, base.offset + off,
                    [base.ap[0], [128, HPC], [1, 64]],
                )


            with (
                tc.tile_pool(name="xt", bufs=1) as xtp,
                tc.tile_pool(name="w", bufs=1) as wps,
                tc.tile_pool(name="psA", bufs=8, space="PSUM") as psA,
            ):
                xt_t = [
                    [xtp.tile([128, 512], f32r, name=f"xt{k}_{s}") for s in range(NSC)]
                    for k in range(NKT)
                ]
                wqk_t = [wps.tile([128, 768], f32r, name=f"wqk{k}") for k in range(NKT)]
                wv_t = [wps.tile([128, 384], f32r, name=f"wv{k}") for k in range(NKT)]
                for k in range(NKT):
                    nc.sync.dma_start(wqk_t[k][:], wqk[k * 128:(k + 1) * 128, :])
                    nc.sync.dma_start(wv_t[k][:], wv[k * 128:(k + 1) * 128, :])
                    for s in range(NSC):
                        nc.sync.dma_start(
                            xt_t[k][s][:], xT[k * 128:(k + 1) * 128, s * 512:(s + 1) * 512]
                        )

                # ---- phase A: qkT[768, 2048] = wqk.T @ xT (+bias) ----
                for jt in range(NJT):
                    for sc in range(NSC):
                        ps = psA.tile([128, 512], f32, tag="a", name="psa")
                        for kt in range(NKT):
                            nc.tensor.matmul(
                                ps[:],
                                wqk_t[kt][:, jt * 128:(jt + 1) * 128],
                                xt_t[kt][sc][:],
                                start=(kt == 0), stop=(kt == NKT - 1),
                            )
                        csl = slice(sc * 512, (sc + 1) * 512)
                        if jt < 3:
                            nc.vector.tensor_scalar_add(
                                qT_t[jt][:, csl], ps[:], bqk_t[:, jt:jt + 1]
                            )
                        else:
                            nc.vector.tensor_scalar_add(
                                kT_t[jt - 3][:, csl], ps[:], bqk_t[:, jt:jt + 1]
                            )

                # ---- phase B: V[2048, 384] = xT.T @ wv (+bias) ----
                for st in range(NST):
                    ps = psA.tile([128, 512], f32, tag="a", name="psb")
                    for kt in range(NKT):
                        nc.tensor.matmul(
                            ps[:, 0:384],
                            xt_t[kt][st // 4][:, (st % 4) * 128:(st % 4 + 1) * 128],
                            wv_t[kt][:],
                            start=(kt == 0), stop=(kt == NKT - 1),
                        )
                    ps384 = bass.AP(ps.tensor, ps.offset, [ps.ap[0], [64, HPC], [1, 64]])
                    bvb = bv_t[:]
                    bv384 = bass.AP(bvb.tensor, bvb.offset, [bvb.ap[0], [64, HPC], [1, 64]])
                    nc.vector.tensor_add(v_strided(st, 0), ps384, bv384)
                    # ones columns: bv*0 + 1 (finite source, f32r-rounded output)
                    nc.vector.tensor_scalar(
                        v_strided(st, 64), bv384, 0.0, 1.0,
                        mybir.AluOpType.mult, mybir.AluOpType.add,
                    )

            # ---- phases C+D share aoT / wp ----
            with tc.tile_pool(name="late", bufs=1) as lp:
                aoT_t = [lp.tile([128, S], f32r, name=f"aoT{i}") for i in range(3)]
                wp_t = [lp.tile([128, D], f32r, name=f"wp{i}") for i in range(3)]
                for i in range(3):
                    nc.sync.dma_start(wp_t[i][:], wp[i * 128:(i + 1) * 128, :])

                # ---- phase C: attention, head-pair parity interleaved ----
                with (
                    tc.tile_pool(name="pt", bufs=12) as ptp,
                    tc.tile_pool(name="psST", bufs=1, space="PSUM") as psST,
                    tc.tile_pool(name="psAV", bufs=1, space="PSUM") as psAV,
                ):

                    for hp in range(3):
                        qt = qT_t[hp]
                        kt3 = kT_t[hp]
                        for ihalf in range(2):
                            i0 = ihalf * 1024
                            avs = {
                                (par, ic): psAV.tile(
                                    [128, 512], f32,
                                    tag=f"av{par}{ic}", name=f"av{par}{ic}",
                                )
                                for par in range(2) for ic in range(2)
                            }
                            for jt in range(NST):
                                jsl = slice(jt * 128, (jt + 1) * 128)
                                sts = {}
                                for par in range(2):
                                    sts[par] = psST.tile(
                                        [128, 1024], f32,
                                        tag=f"st{par}", name=f"st{par}",
                                    )
                                # strict parity alternation for the K=64 matmuls
                                for ic in range(2):
                                    for par in range(2):
                                        psl = slice(par * 64, par * 64 + 64)
                                        nc.tensor.matmul(
                                            sts[par][:, ic * 512:(ic + 1) * 512],
                                            kt3[psl, jsl],
                                            qt[psl, i0 + ic * 512:i0 + (ic + 1) * 512],
                                            start=True, stop=True,
                                        )
                                pts = {}
                                for par in range(2):
                                    pt = ptp.tile([128, 1024], f32r, tag="pt", name="pt")
                                    nc.scalar.activation(pt[:], sts[par][:], Exp)
                                    pts[par] = pt
                                for par in range(2):
                                    h = 2 * hp + par
                                    for ic in range(2):
                                        nc.tensor.matmul(
                                            avs[par, ic][:],
                                            v_t[jt][:, h * 128:(h + 1) * 128],
                                            pts[par][:, ic * 512:(ic + 1) * 512],
                                            start=(jt == 0), stop=(jt == NST - 1),
                                        )
                            for par in range(2):
                                for ic in range(2):
                                    av = avs[par, ic]
                                    rec = recp.tile([128, 512], f32, tag="rec", name="rec")
                                    nc.vector.reciprocal(rec[64:128, :], av[64:128, :])
                                    dst = aoT_t[hp][
                                        par * 64:par * 64 + 64,
                                        i0 + ic * 512:i0 + (ic + 1) * 512,
                                    ]
                                    nc.vector.tensor_mul(dst, av[0:64, :], rec[64:128, :])

                # ---- phase D: outT[768, 2048] = wp.T @ aoT ----
                with tc.tile_pool(name="psO", bufs=8, space="PSUM") as psO:
                    for jt2 in range(NJT):
                        for ic in range(NSC):
                            ps = psO.tile([128, 512], f32, tag="o", name="pso")
                            for kt3 in range(3):
                                nc.tensor.matmul(
                                    ps[:],
                                    wp_t[kt3][:, jt2 * 128:(jt2 + 1) * 128],
                                    aoT_t[kt3][:, ic * 512:(ic + 1) * 512],
                                    start=(kt3 == 0), stop=(kt3 == 2),
                                )
                            o = stg.tile([128, 512], f32, tag="os", name="os")
                            nc.vector.tensor_copy(o[:], ps[:])
                            nc.sync.dma_start(
                                outT[jt2 * 128:(jt2 + 1) * 128, ic * 512:(ic + 1) * 512],
                                o[:],
                            )

    nc.compile()
    return nc


def _prep_core_inputs(x, w_qkv, b_qkv, w_proj, b, g):
    q0 = g * HPC * DH            # start col of this group's q block
    qs = slice(q0, q0 + 384)
    ks = slice(768 + q0, 768 + q0 + 384)
    vs = slice(1536 + q0, 1536 + q0 + 384)

    xTc = _round_fp32r(x[b].T)
    wqk_h = np.concatenate([w_qkv[:, qs] * 0.125, w_qkv[:, ks]], axis=1)
    wqk_h = _round_fp32r(wqk_h)
    bqk_flat = np.concatenate([b_qkv[qs] * 0.125, b_qkv[ks]])
    bqk_h = np.ascontiguousarray(bqk_flat.reshape(NJT, 128).T, dtype=np.float32)
    wv_h = _round_fp32r(w_qkv[:, vs])
    bv_h = np.ascontiguousarray(
        np.broadcast_to(b_qkv[vs].astype(np.float32), (128, 384))
    )
    wp_h = _round_fp32r(w_proj[g * 384:(g + 1) * 384, :])
    return {"xT": xTc, "wqk": wqk_h, "bqk": bqk_h, "wv": wv_h, "bv": bv_h, "wp": wp_h}


def kernel(x, w_qkv, b_qkv, w_proj, b_proj):
    from concourse.bass_utils import run_bass_kernel_spmd

    x = np.asarray(x, dtype=np.float32)
    w_qkv = np.asarray(w_qkv, dtype=np.float32)
    b_qkv = np.asarray(b_qkv, dtype=np.float32)
    w_proj = np.asarray(w_proj, dtype=np.float32)
    b_proj = np.asarray(b_proj, dtype=np.float32)

    if "nc" not in _NC_CACHE:
        _NC_CACHE["nc"] = _build_nc()
    nc = _NC_CACHE["nc"]

    in_maps = [
        _prep_core_inputs(x, w_qkv, b_qkv, w_proj, core // 2, core % 2)
        for core in range(8)
    ]
    res = run_bass_kernel_spmd(nc, in_maps, core_ids=list(range(8)))

    out = np.empty((B, S, D), dtype=np.float32)
    for b in range(B):
        t0 = res.results[2 * b]["outT"]
        t1 = res.results[2 * b + 1]["outT"]
        out[b] = (t0.T + t1.T) + b_proj
    return out



# revision 28
# speedup vs baseline: 532.0849x; 532.0849x over previous
"""Multi-head attention (B=4, S=2048, D=768, H=12, d=64) on 8 Trainium2 cores.

Sharding: core (b, g) = batch b in [0,4), head-group g in [0,2) -- 6 heads
each (3 head-pairs). Each core computes the qkv projection for its heads,
attention, and a partial (transposed) output projection; the host sums the
two head-group partials per batch and adds b_proj.

v2: single software-pipelined schedule. The scalar engine's exp stream
(192 x ACTIVATE [128,1024], ~191us) is the critical path; all projection
matmuls (qk-proj "A", v-proj "B", out-proj "D") are interleaved into the
tensor-engine stall gaps of the attention phase "C" instead of running as
separate serial phases. PSUM: 4 banks ST ping-pong + 2 banks AV + 2 banks
filler. P (exp output) and V are bf16 (halves SBUF, same matmul rate);
scores stay f32r. Softmax denominators via the ones-columns trick ride in
the AV matmul's free M dimension; normalize uses reciprocal_approx_fast.

Per chunk of 512 queries, per key-tile jt (128 keys):
  ST[:, 0:512]   = kT[0:64, jt].T   @ qT[0:64, chunk]    (PE row-tile T0)
  ST[:, 512:1024]= kT[64:128, jt].T @ qT[64:128, chunk]  (PE row-tile T8)
  pt = exp(ST)                                  (ACT, PSUM->SBUF, bf16)
  av0 += v[jt, h0].T @ pt[:, 0:512]   av1 += v[jt, h1].T @ pt[:, 512:1024]
"""
import numpy as np

B, S, D = 4, 2048, 768
H, DH = 12, 64
HPC = 6          # heads per core
NKT = D // 128   # 6 contraction tiles of 128
NSC = S // 512   # 4 column chunks of 512
NST = S // 128   # 16 key tiles of 128
NJT = 6          # qk projection output row tiles (768/128)
VW = HPC * 128   # v tile width: per head [v_h (64) | ones (64)]

_NC_CACHE = {}


def _round_fp32r(x):
    """Round fp32 to the fp32r grid (E8M11: low 12 mantissa bits zero, RNE)."""
    x = np.ascontiguousarray(x, dtype=np.float32)
    u = x.view(np.uint32).astype(np.uint64)
    u = (u + 0x7FF + ((u >> 12) & 1)) & 0xFFFFF000
    return u.astype(np.uint32).view(np.float32)


def _build_nc(reps=1, debug_dumps=False, interleave=True):
    import contextlib

    import concourse.bass as bass
    import concourse.mybir as mybir
    import concourse.tile as tile
    from concourse import bacc

    f32r = mybir.dt.float32r
    f32 = mybir.dt.float32
    bf16 = mybir.dt.bfloat16
    Exp = mybir.ActivationFunctionType.Exp

    nc = bacc.Bacc("TRN2", target_bir_lowering=False, debug=False)
    xT = nc.dram_tensor("xT", [D, S], f32r, kind="ExternalInput").ap()
    wqk = nc.dram_tensor("wqk", [D, 768], f32r, kind="ExternalInput").ap()
    bqk = nc.dram_tensor("bqk", [128, NJT], f32, kind="ExternalInput").ap()
    wv = nc.dram_tensor("wv", [D, 384], f32r, kind="ExternalInput").ap()
    bv = nc.dram_tensor("bv", [128, 384], f32, kind="ExternalInput").ap()
    wp = nc.dram_tensor("wp", [384, D], f32r, kind="ExternalInput").ap()
    outT = nc.dram_tensor("outT", [D, S], f32, kind="ExternalOutput").ap()
    if debug_dumps:
        dbg = {
            name: nc.dram_tensor(name, shp, dt, kind="ExternalOutput").ap()
            for name, shp, dt in [
                ("dbg_qT", [3 * 128, S], mybir.dt.float32r),
                ("dbg_kT", [3 * 128, S], mybir.dt.float32r),
                ("dbg_v", [NST * 128, VW], mybir.dt.bfloat16),
                ("dbg_aoT", [3 * 128, S], mybir.dt.float32r),
                ("dbg_st", [128, 1024], mybir.dt.float32),
                ("dbg_pt", [128, 1024], mybir.dt.bfloat16),
                ("dbg_av", [256, 512], mybir.dt.float32),
            ]
        }

    with tile.TileContext(nc) as tc:
      with (tc.For_i(0, reps, 1) if reps != 1 else contextlib.nullcontext()):
        with (
            tc.tile_pool(name="persist", bufs=1) as pp,
            tc.tile_pool(name="pt", bufs=3) as ptp,
            tc.tile_pool(name="rec", bufs=2) as recp,
            tc.tile_pool(name="ostg", bufs=4) as ostg,
            tc.tile_pool(name="dbgp", bufs=1) as dbgp,
            tc.tile_pool(name="psST", bufs=2, space="PSUM") as psST,
            tc.tile_pool(name="psAV", bufs=1, space="PSUM") as psAV,
            tc.tile_pool(name="psF", bufs=2, space="PSUM") as psF,
        ):
            # ---- persistent SBUF ----
            qT_t = [pp.tile([128, S], f32r, name=f"qT{i}") for i in range(3)]
            kT_t = [pp.tile([128, S], f32r, name=f"kT{i}") for i in range(3)]
            v_t = [pp.tile([128, VW], bf16, name=f"v{i}") for i in range(NST)]
            aoT_t = [pp.tile([128, S], f32r, name=f"aoT{i}") for i in range(3)]
            xt_t = [
                [pp.tile([128, 512], f32r, name=f"xt{k}_{s}") for s in range(NSC)]
                for k in range(NKT)
            ]
            wqk_t = [pp.tile([128, 768], f32r, name=f"wqk{k}") for k in range(NKT)]
            wv_t = [pp.tile([128, 384], f32r, name=f"wv{k}") for k in range(NKT)]
            wp_t = [pp.tile([128, D], f32r, name=f"wp{i}") for i in range(3)]
            bqk_t = pp.tile([128, NJT], f32, name="bqk")
            bv_t = pp.tile([128, 384], f32, name="bv")

            # ---- DMA, ordered for earliest attention start ----
            # first xt column + the hp0 wqk columns + wv, then the rest.
            nc.sync.dma_start(bqk_t[:], bqk)
            nc.sync.dma_start(bv_t[:], bv)
            for k in range(NKT):
                nc.sync.dma_start(
                    xt_t[k][0][:], xT[k * 128:(k + 1) * 128, 0:512]
                )
            for col in (0, 3):  # q col hp0, k col hp0
                for k in range(NKT):
                    nc.sync.dma_start(
                        wqk_t[k][:, col * 128:(col + 1) * 128],
                        wqk[k * 128:(k + 1) * 128, col * 128:(col + 1) * 128],
                    )
            for k in range(NKT):
                nc.sync.dma_start(wv_t[k][:], wv[k * 128:(k + 1) * 128, :])
            for s in range(1, NSC):
                for k in range(NKT):
                    nc.sync.dma_start(
                        xt_t[k][s][:], xT[k * 128:(k + 1) * 128, s * 512:(s + 1) * 512]
                    )
            for col in (1, 4, 2, 5):  # q/k cols for hp1, hp2
                for k in range(NKT):
                    nc.sync.dma_start(
                        wqk_t[k][:, col * 128:(col + 1) * 128],
                        wqk[k * 128:(k + 1) * 128, col * 128:(col + 1) * 128],
                    )
            for i in range(3):
                nc.sync.dma_start(wp_t[i][:], wp[i * 128:(i + 1) * 128, :])

            # ones columns of v tiles (denominator trick), constant.
            # plain 2-dim slices only: hand-built multi-dim APs are not seen
            # by Tile's dependency tracking, which matters now that v writes
            # race with interleaved attention reads.
            for st in range(NST):
                for h in range(HPC):
                    nc.vector.tensor_scalar(
                        v_t[st][:, h * 128 + 64:h * 128 + 128],
                        bv_t[:, 0:64], 0.0, 1.0,
                        mybir.AluOpType.mult, mybir.AluOpType.add,
                    )

            # ---- filler emission machinery ----
            # Each filler is a closure emitting one PSUM group (matmuls +
            # evacuation). Emitted between attention iterations to fill
            # tensor-engine stall gaps under the scalar-bound exp stream.
            def emit_A(hp, which, sc):
                # qk-projection: one [128,512] tile of q (which=0) or k (=1)
                col = hp + 3 * which
                ps = psF.tile([128, 512], f32, tag="fill", name="psA")
                for kt in range(NKT):
                    nc.tensor.matmul(
                        ps[:],
                        wqk_t[kt][:, col * 128:(col + 1) * 128],
                        xt_t[kt][sc][:],
                        start=(kt == 0), stop=(kt == NKT - 1),
                    )
                dst = (qT_t if which == 0 else kT_t)[hp][:, sc * 512:(sc + 1) * 512]
                nc.vector.tensor_scalar_add(dst, ps[:], bqk_t[:, col:col + 1])

            def emit_B(st):
                # v-projection for key tile st, all 6 heads
                ps = psF.tile([128, 512], f32, tag="fill", name="psB")
                for kt in range(NKT):
                    nc.tensor.matmul(
                        ps[:, 0:384],
                        xt_t[kt][st // 4][:, (st % 4) * 128:(st % 4 + 1) * 128],
                        wv_t[kt][:],
                        start=(kt == 0), stop=(kt == NKT - 1),
                    )
                # plain-slice evacuation per head (dependency-tracked)
                for h in range(HPC):
                    nc.vector.tensor_add(
                        v_t[st][:, h * 128:h * 128 + 64],
                        ps[:, h * 64:(h + 1) * 64],
                        bv_t[:, h * 64:(h + 1) * 64],
                    )

            def emit_D(jt2, ic):
                # out-projection tile [128,512]: contraction over all 3 aoT
                ps = psF.tile([128, 512], f32, tag="fill", name="psD")
                for kt3 in range(3):
                    nc.tensor.matmul(
                        ps[:],
                        wp_t[kt3][:, jt2 * 128:(jt2 + 1) * 128],
                        aoT_t[kt3][:, ic * 512:(ic + 1) * 512],
                        start=(kt3 == 0), stop=(kt3 == 2),
                    )
                o = ostg.tile([128, 512], f32, tag="os", name="os")
                nc.vector.tensor_copy(o[:], ps[:])
                nc.sync.dma_start(
                    outT[jt2 * 128:(jt2 + 1) * 128, ic * 512:(ic + 1) * 512],
                    o[:],
                )

            # lead-in: k for hp0 (all S), q for hp0 chunks 0-1, first B tiles
            if interleave:
                for sc in range(NSC):
                    emit_A(0, 1, sc)
                emit_A(0, 0, 0)
                for st in range(8):
                    emit_B(st)
                emit_A(0, 0, 1)
            else:
                for hp_ in range(3):
                    for w_ in (0, 1):
                        for sc in range(NSC):
                            emit_A(hp_, w_, sc)
                for st in range(NST):
                    emit_B(st)

            # filler placement: (hp, chunk, jt) -> list of closures.
            # B tiles land a few iterations ahead of the attention read of
            # the same key tile; q/k projection columns land a chunk ahead
            # of their first use.
            filler_at = {}

            def put(hp, chunk, jt, fn):
                if not interleave:
                    return
                filler_at.setdefault((hp, chunk, jt), []).append(fn)

            for st in range(8, NST):           # B st tiles, 4-ahead in chunk0
                put(0, 0, st - 4, lambda st=st: emit_B(st))
            put(0, 1, 0, lambda: emit_A(0, 0, 2))
            put(0, 1, 4, lambda: emit_A(0, 0, 3))
            put(0, 1, 8, lambda: emit_A(1, 1, 0))
            put(0, 1, 12, lambda: emit_A(1, 1, 1))
            put(0, 2, 0, lambda: emit_A(1, 1, 2))
            put(0, 2, 4, lambda: emit_A(1, 1, 3))
            put(0, 2, 8, lambda: emit_A(1, 0, 0))
            put(0, 2, 12, lambda: emit_A(1, 0, 1))
            put(0, 3, 0, lambda: emit_A(1, 0, 2))
            put(0, 3, 8, lambda: emit_A(1, 0, 3))
            for sc in range(NSC):              # hp2 projections during hp1
                put(1, sc, 2, lambda sc=sc: emit_A(2, 1, sc))
                put(1, sc, 9, lambda sc=sc: emit_A(2, 0, sc))

            # ---- attention + interleaved fillers ----
            for hp in range(3):
                for chunk in range(NSC):
                    q0 = chunk * 512
                    avs = {
                        par: psAV.tile(
                            [128, 512], f32, tag=f"av{par}", name=f"av{par}"
                        )
                        for par in range(2)
                    }
                    for jt in range(NST):
                        jsl = slice(jt * 128, (jt + 1) * 128)
                        st_ps = psST.tile([128, 1024], f32, tag="st", name="st")
                        for par in range(2):
                            psl = slice(par * 64, par * 64 + 64)
                            nc.tensor.matmul(
                                st_ps[:, par * 512:(par + 1) * 512],
                                kT_t[hp][psl, jsl],
                                qT_t[hp][psl, q0:q0 + 512],
                                start=True, stop=True,
                            )
                        pt = ptp.tile([128, 1024], bf16, tag="pt", name="pt")
                        nc.scalar.activation(pt[:], st_ps[:], Exp)
                        for par in range(2):
                            h = 2 * hp + par
                            nc.tensor.matmul(
                                avs[par][:],
                                v_t[jt][:, h * 128:(h + 1) * 128],
                                pt[:, par * 512:(par + 1) * 512],
                                start=(jt == 0), stop=(jt == NST - 1),
                            )
                        if debug_dumps and hp == 2 and chunk == 3 and jt == 15:
                            stg = dbgp.tile([128, 1024], f32, tag="dbgst",
                                            name="dbgst")
                            nc.vector.tensor_copy(stg[:], st_ps[:])
                            nc.sync.dma_start(dbg["dbg_st"], stg[:])
                            nc.sync.dma_start(dbg["dbg_pt"], pt[:])
                        for fn in filler_at.get((hp, chunk, jt), ()):
                            fn()
                    # normalize chunk -> aoT
                    if debug_dumps and hp == 2 and chunk == 3:
                        for par in range(2):
                            stg = dbgp.tile([128, 512], f32, tag="dbgav",
                                            name="dbgav")
                            nc.vector.tensor_copy(stg[:], avs[par][:])
                            nc.sync.dma_start(
                                dbg["dbg_av"][par * 128:(par + 1) * 128, :],
                                stg[:],
                            )
                    for par in range(2):
                        av = avs[par]
                        rec = recp.tile([128, 512], f32, tag="rec", name="rec")
                        # full 128 partitions: the custom-DVE op mis-executes
                        # on a base_partition!=0 slice; rows 0:64 (reciprocal
                        # of numerators) are computed but unused.
                        nc.vector.reciprocal_approx_fast(rec[:], av[:])
                        dst = aoT_t[hp][
                            par * 64:par * 64 + 64, q0:q0 + 512,
                        ]
                        nc.vector.tensor_mul(dst, av[0:64, :], rec[64:128, :])
                    # during the last head-pair, emit out-projection slabs for
                    # columns whose aoT is already complete
                    if interleave and hp == 2 and chunk >= 1:
                        for jt2 in range(NJT):
                            emit_D(jt2, chunk - 1)

            # tail: last out-projection slab
            if interleave:
                for jt2 in range(NJT):
                    emit_D(jt2, NSC - 1)
            else:
                for ic in range(NSC):
                    for jt2 in range(NJT):
                        emit_D(jt2, ic)

            if debug_dumps:
                for i in range(3):
                    for src, name in ((qT_t, "dbg_qT"), (kT_t, "dbg_kT"),
                                      (aoT_t, "dbg_aoT")):
                        nc.sync.dma_start(
                            dbg[name][i * 128:(i + 1) * 128, :], src[i][:]
                        )
                for st in range(NST):
                    nc.sync.dma_start(
                        dbg["dbg_v"][st * 128:(st + 1) * 128, :], v_t[st][:]
                    )

    nc.compile()
    return nc


def _prep_core_inputs(x, w_qkv, b_qkv, w_proj, b, g):
    q0 = g * HPC * DH            # start col of this group's q block
    qs = slice(q0, q0 + 384)
    ks = slice(768 + q0, 768 + q0 + 384)
    vs = slice(1536 + q0, 1536 + q0 + 384)

    xTc = _round_fp32r(x[b].T)
    wqk_h = np.concatenate([w_qkv[:, qs] * 0.125, w_qkv[:, ks]], axis=1)
    wqk_h = _round_fp32r(wqk_h)
    bqk_flat = np.concatenate([b_qkv[qs] * 0.125, b_qkv[ks]])
    bqk_h = np.ascontiguousarray(bqk_flat.reshape(NJT, 128).T, dtype=np.float32)
    wv_h = _round_fp32r(w_qkv[:, vs])
    bv_h = np.ascontiguousarray(
        np.broadcast_to(b_qkv[vs].astype(np.float32), (128, 384))
    )
    wp_h = _round_fp32r(w_proj[g * 384:(g + 1) * 384, :])
    return {"xT": xTc, "wqk": wqk_h, "bqk": bqk_h, "wv": wv_h, "bv": bv_h, "wp": wp_h}


def kernel(x, w_qkv, b_qkv, w_proj, b_proj):
    from concourse.bass_utils import run_bass_kernel_spmd

    x = np.asarray(x, dtype=np.float32)
    w_qkv = np.asarray(w_qkv, dtype=np.float32)
    b_qkv = np.asarray(b_qkv, dtype=np.float32)
    w_proj = np.asarray(w_proj, dtype=np.float32)
    b_proj = np.asarray(b_proj, dtype=np.float32)

    if "nc" not in _NC_CACHE:
        _NC_CACHE["nc"] = _build_nc()
    nc = _NC_CACHE["nc"]

    in_maps = [
        _prep_core_inputs(x, w_qkv, b_qkv, w_proj, core // 2, core % 2)
        for core in range(8)
    ]
    res = run_bass_kernel_spmd(nc, in_maps, core_ids=list(range(8)))

    out = np.empty((B, S, D), dtype=np.float32)
    for b in range(B):
        t0 = res.results[2 * b]["outT"]
        t1 = res.results[2 * b + 1]["outT"]
        out[b] = (t0.T + t1.T) + b_proj
    return out
